# revision 33
# baseline (speedup 1.0000x reference)
"""AnyPrecisionLinear (4-bit LUT-quantized linear) Trainium2 kernel, 8-core SPMD.

y[b,s,o] = sum_i x[b,s,i] * lut[o, code[o,i]] + bias[o]
code assembled MSB-first from bitplanes 0..3 of qweight.

Sharding (column-parallel, per hint): out_features padded 11008->11264 and
split 1408 per core; x replicated; no collectives. Output gathered on host.

Per-core pipeline (o-tiles processed in groups so the GEMM of group g
overlaps the dequant of group g+1 in the static Tile schedule):
  dequant per o-tile (128 rows):
    - bit tiles via fused bitwise shift+and tensor_scalar (int32, DVE)
    - fp16 converts of the bit tiles (DVE) for 2x-mode tree ops
    - 8 codebook "leaves" t_k = b3*d_k + lut[:,2k] on the Scalar engine
      (per-partition fp32 scale/bias APs)
    - 7-op copy_predicated binary select tree (DVE, fp16 2x mode)
    - TensorEngine 128x128 transposes into WT[i',o] (GEMM stationary layout)
  GEMM per group: for each 256-token block accumulate 32 K-tiles in PSUM;
  epilogue adds bias + casts to fp16 on the Scalar engine.

The in-feature axis is processed in a permuted order i' = jj*128 + w
(i = 32w + jj) making bit extraction and K-tiling dense; the host applies
the same permutation to x (contraction order is free).
"""

import numpy as np

IN = 4096
O_FULL = 11008
NCORES = 8
O_PAD = 11264          # 8 * 11 * 128
O_SH = O_PAD // NCORES  # 1408
OT = O_SH // 128        # 11 o-tiles
KT = IN // 128          # 32 k-tiles
T = 4096                # tokens
TBLK = 256
NTB = T // TBLK         # 16 token blocks
NQ = 8                  # bit-positions (jj) per dequant pass
NPASS = KT // NQ        # 4 passes per o-tile
GROUPS = [2, 3, 3, 3]   # o-tile pipeline groups (GEMM g overlaps dequant g+1)

# schedule/config knobs (read at build time; timeline sweeps override these)
CONFIG = {
    "mask_conv": None,     # None | "int16": convert b0..b2 masks (walrus
                           # requires int mask dtypes for CopyPredicated)
    "bpool_bufs": 2,
    "fpool_bufs": 2,
    "tpool_bufs": 2,
    "x_halves": True,
    "groups": GROUPS,
    "leaf_dve": 8,       # how many of the 8 leaves run on DVE tensor_scalar
    "ps_mm_bufs": 4,
    "ps_tr_bufs": 2,
    "weave": False,       # fine-grained dq/gemm interleave in program order
    "loop_n": None,
    # timing-only ablations (wrong results, same structure):
    "nq": 8,              # bit-positions per dequant pass
    "tblk": 256,          # token block (512 -> kt-quarter x streaming)
    "fused_tr": True,     # per-pass single PSUM bank + one fused eviction
    "x_dma_only": False,  # ablation: gemm_group loads x slabs only
    "out_quad": False,    # stage 4 token-blocks per output DMA
    "skip_tree": False,
    "skip_gemm": False,
    "skip_dequant": False,
}

_PROGRAM = None


def _build_program():
    import concourse.mybir as mybir
    import concourse.tile as tile
    from concourse import bacc
    from concourse.masks import make_identity
    from contextlib import ExitStack

    nc = bacc.Bacc("TRN2", target_bir_lowering=False, debug=False,
                   num_devices=NCORES)

    qw_e = nc.dram_tensor("qw", [4, O_SH, 128], mybir.dt.int32,
                          kind="ExternalInput")
    lut_e = nc.dram_tensor("lut", [O_SH, 16], mybir.dt.float16,
                           kind="ExternalInput")
    bias_e = nc.dram_tensor("bias", [O_SH, 1], mybir.dt.float16,
                            kind="ExternalInput")
    tblk = CONFIG["tblk"]
    ntb = T // tblk
    # x pre-tiled on host to [tb, p, kt, u] so each token-block slab DMA
    # reads 16KB contiguous per partition (vs 512B strided chunks)
    xt_e = nc.dram_tensor("xt", [ntb, 128, KT, tblk], mybir.dt.float16,
                          kind="ExternalInput")
    out_e = nc.dram_tensor("out", [O_SH, T], mybir.dt.float16,
                           kind="ExternalOutput")

    with tile.TileContext(nc) as tc:
        ctx = ExitStack()
        singles = ctx.enter_context(tc.tile_pool(name="singles", bufs=1))
        qpool = ctx.enter_context(tc.tile_pool(name="qpool", bufs=1))
        bpool = ctx.enter_context(tc.tile_pool(name="bpool",
                                               bufs=CONFIG["bpool_bufs"]))
        fpool = ctx.enter_context(tc.tile_pool(name="fpool",
                                               bufs=CONFIG["fpool_bufs"]))
        tpool = ctx.enter_context(tc.tile_pool(name="tpool",
                                               bufs=CONFIG["tpool_bufs"]))
        wpool = ctx.enter_context(tc.tile_pool(name="wpool", bufs=1))
        xpool = ctx.enter_context(tc.tile_pool(
            name="xpool", bufs=3 if CONFIG["x_halves"] else 2))
        opool = ctx.enter_context(tc.tile_pool(name="opool", bufs=4))
        ps_tr = ctx.enter_context(tc.tile_pool(
            name="ps_tr", bufs=CONFIG["ps_tr_bufs"], space="PSUM"))
        ps_mm = ctx.enter_context(tc.tile_pool(
            name="ps_mm",
            bufs=2 if CONFIG["tblk"] == 512 else CONFIG["ps_mm_bufs"],
            space="PSUM"))

        # --- constants -----------------------------------------------------
        ident = singles.tile([128, 128], mybir.dt.float16, name="ident")
        make_identity(nc, ident[:])

        lut_sb = singles.tile([128, OT, 16], mybir.dt.float16, name="lut_sb")
        nc.sync.dma_start(
            out=lut_sb[:],
            in_=lut_e.ap().rearrange("(ot p) c -> p ot c", p=128))
        lut32 = singles.tile([128, OT, 16], mybir.dt.float32, name="lut32")
        nc.vector.tensor_copy(out=lut32[:], in_=lut_sb[:])
        dq = singles.tile([128, OT, 8], mybir.dt.float32, name="dq")
        nc.vector.tensor_tensor(out=dq[:], in0=lut32[:, :, 1::2],
                                in1=lut32[:, :, 0::2],
                                op=mybir.AluOpType.subtract)

        bias_sb = singles.tile([128, OT], mybir.dt.float16, name="bias_sb")
        nc.sync.dma_start(
            out=bias_sb[:],
            in_=bias_e.ap().rearrange("(ot p) c -> p (ot c)", p=128))
        bias32 = singles.tile([128, OT], mybir.dt.float32, name="bias32")
        nc.vector.tensor_copy(out=bias32[:], in_=bias_sb[:])

        # persistent transposed weights, one contiguous tile per o-tile:
        # wto[ot][:, kt*128:(kt+1)*128] is the [128 i', 128 o] fp16 k-tile
        if CONFIG["fused_tr"]:
            wto = [wpool.tile([128, KT * 128], mybir.dt.float16,
                              name=f"wto_{ot}", tag=f"wto_{ot}")
                   for ot in range(OT)]
            wt = [[wto[ot][:, kt * 128:(kt + 1) * 128] for kt in range(KT)]
                  for ot in range(OT)]
            if CONFIG["skip_dequant"]:
                for ot in range(OT):
                    nc.vector.memset(wto[ot][:], 0.0)
        else:
            wt = [[wpool.tile([128, 128], mybir.dt.float16,
                              name=f"wt_{ot}_{kt}", tag=f"wt_{ot}_{kt}")
                   for kt in range(KT)] for ot in range(OT)]
            if CONFIG["skip_dequant"]:
                for ot in range(OT):
                    for kt in range(KT):
                        nc.vector.memset(wt[ot][kt][:], 0.0)


        def dequant_otile(ot):
            if CONFIG["skip_dequant"]:
                return
            qt = [qpool.tile([128, 128], mybir.dt.int32, name=f"qt{p}",
                             tag=f"qt{p}") for p in range(4)]
            for p in range(4):
                nc.sync.dma_start(out=qt[p][:],
                                  in_=qw_e[p, ot * 128:(ot + 1) * 128, :])
            nq = CONFIG["nq"]
            for ps in range(KT // nq):
                jj0 = ps * nq
                # int32 0/1 bit tiles via fused bitwise shift+and (bitwise
                # TensorScalar cannot cast, so extraction stays int32)
                bt = [bpool.tile([128, nq * 128], mybir.dt.int32,
                                 name=f"bt{p}", tag=f"bt{p}")
                      for p in range(3)]
                for j in range(nq):
                    for p in range(3):
                        # all-DVE: GPSIMD shares the DVE SBUF port (exclusive
                        # lock), so offloading there serializes instead
                        nc.vector.tensor_scalar(
                            out=bt[p][:, j * 128:(j + 1) * 128],
                            in0=qt[p][:],
                            scalar1=31 - (jj0 + j),
                            scalar2=1,
                            op0=mybir.AluOpType.logical_shift_right,
                            op1=mybir.AluOpType.bitwise_and,
                        )
                # b3 (LSB plane) extracted straight to exact fp16 0/1 for the
                # leaves: shift-left (bitwise, no cast), then sign-compare
                # (arith, cast allowed)
                b3i = bpool.tile([128, nq * 128], mybir.dt.int32,
                                 name="b3i", tag="b3i")
                for j in range(nq):
                    nc.vector.tensor_scalar(
                        out=b3i[:, j * 128:(j + 1) * 128],
                        in0=qt[3][:], scalar1=jj0 + j, scalar2=None,
                        op0=mybir.AluOpType.logical_shift_left,
                        op1=mybir.AluOpType.bypass)
                b3f = fpool.tile([128, nq * 128], mybir.dt.float16,
                                 name="b3f", tag="b3f")
                nc.vector.tensor_scalar(
                    out=b3f[:], in0=b3i[:], scalar1=0, scalar2=None,
                    op0=mybir.AluOpType.is_lt, op1=mybir.AluOpType.bypass)
                msk = [bt[p] for p in range(3)]
                if CONFIG["mask_conv"] == "int16":
                    for p in range(3):
                        m16 = fpool.tile([128, nq * 128], mybir.dt.int16,
                                         name=f"m16_{p}", tag=f"m16_{p}")
                        nc.vector.tensor_copy(out=m16[:], in_=bt[p][:])
                        msk[p] = m16

                tk = [tpool.tile([128, nq * 128], mybir.dt.float16,
                                 name=f"tk{k}", tag=f"tk{k}")
                      for k in range(8)]
                for k in range(8):
                    if k < CONFIG["leaf_dve"]:
                        nc.vector.tensor_scalar(
                            out=tk[k][:], in0=b3f[:],
                            scalar1=dq[:, ot, k:k + 1],
                            scalar2=lut32[:, ot, 2 * k:2 * k + 1],
                            op0=mybir.AluOpType.mult,
                            op1=mybir.AluOpType.add,
                        )
                    else:
                        nc.scalar.activation(
                            out=tk[k][:], in_=b3f[:],
                            func=mybir.ActivationFunctionType.Identity,
                            bias=lut32[:, ot, 2 * k:2 * k + 1],
                            scale=dq[:, ot, k:k + 1],
                        )
                if not CONFIG["skip_tree"]:
                    for j in range(4):
                        nc.vector.copy_predicated(out=tk[2 * j][:],
                                                  mask=msk[2][:],
                                                  data=tk[2 * j + 1][:])
                    nc.vector.copy_predicated(out=tk[0][:], mask=msk[1][:],
                                              data=tk[2][:])
                    nc.vector.copy_predicated(out=tk[4][:], mask=msk[1][:],
                                              data=tk[6][:])
                    nc.vector.copy_predicated(out=tk[0][:], mask=msk[0][:],
                                              data=tk[4][:])

                if CONFIG["fused_tr"]:
                    # all nq transposes land in one PSUM bank; one eviction
                    pt = ps_tr.tile([128, nq * 128], mybir.dt.float16,
                                    name="pt", tag="pt")
                    for s in range(nq):
                        nc.tensor.transpose(pt[:, s * 128:(s + 1) * 128],
                                            tk[0][:, s * 128:(s + 1) * 128],
                                            ident[:])
                    nc.scalar.copy(
                        out=wto[ot][:, jj0 * 128:(jj0 + nq) * 128],
                        in_=pt[:])
                else:
                    for s in range(nq):
                        pt = ps_tr.tile([128, 128], mybir.dt.float16,
                                        name="pt", tag="pt")
                        nc.tensor.transpose(pt[:],
                                            tk[0][:, s * 128:(s + 1) * 128],
                                            ident[:])
                        nc.scalar.copy(out=wt[ot][jj0 + s][:], in_=pt[:])

        ostage = {}

        def gemm_group(ots, tbs=None):
            if CONFIG["skip_gemm"]:
                return
            if tblk == 512:
                gemm_group_512(ots, tbs)
                return
            nh = 2 if CONFIG["x_halves"] else 1
            KH = KT // nh
            for tb in (range(ntb) if tbs is None else tbs):
                xs = [xpool.tile([128, KH, tblk], mybir.dt.float16,
                                 name=f"xs{h}", tag="xs") for h in range(nh)]
                for h in range(nh):
                    nc.sync.dma_start(
                        out=xs[h][:],
                        in_=xt_e[tb, :, h * KH:(h + 1) * KH, :])
                if CONFIG["x_dma_only"]:
                    continue
                for ot in ots:
                    pm = ps_mm.tile([128, tblk], mybir.dt.float32, name="pm",
                                    tag="pm")
                    for kt in range(KT):
                        nc.tensor.matmul(pm[:], lhsT=wt[ot][kt][:],
                                         rhs=xs[kt // KH][:, kt % KH, :],
                                         start=(kt == 0), stop=(kt == KT - 1))
                    ob = opool.tile([128, tblk], mybir.dt.float16,
                                    name="ob", tag="ob")
                    nc.scalar.activation(
                        out=ob[:], in_=pm[:],
                        func=mybir.ActivationFunctionType.Identity,
                        bias=bias32[:, ot:ot + 1], scale=1.0)
                    nc.sync.dma_start(
                        out=out_e[ot * 128:(ot + 1) * 128,
                                  tb * tblk:(tb + 1) * tblk],
                        in_=ob[:])

        def gemm_group_512(ots, tbs):
            # N=512 matmuls with x streamed in kt-quarters; per-ot PSUM
            # accumulators (<=3 in a group) live across the quarters
            NQX = 4
            KQ = KT // NQX
            for tb in (range(ntb) if tbs is None else tbs):
                pms = {}
                for q in range(NQX):
                    xq = xpool.tile([128, KQ, tblk], mybir.dt.float16,
                                    name="xq", tag="xs")
                    nc.sync.dma_start(out=xq[:],
                                      in_=xt_e[tb, :, q * KQ:(q + 1) * KQ, :])
                    if CONFIG["x_dma_only"]:
                        continue
                    for ot in ots:
                        if q == 0:
                            pms[ot] = ps_mm.tile(
                                [128, tblk], mybir.dt.float32,
                                name=f"pm{ot % 3}", tag=f"pm{ot % 3}")
                        for k in range(KQ):
                            kt = q * KQ + k
                            nc.tensor.matmul(
                                pms[ot][:], lhsT=wt[ot][kt][:],
                                rhs=xq[:, k, :],
                                start=(kt == 0), stop=(kt == KT - 1))
                if CONFIG["x_dma_only"]:
                    continue
                for ot in ots:
                    ob = opool.tile([128, tblk], mybir.dt.float16,
                                    name="ob", tag="ob")
                    nc.scalar.activation(
                        out=ob[:], in_=pms[ot][:],
                        func=mybir.ActivationFunctionType.Identity,
                        bias=bias32[:, ot:ot + 1], scale=1.0)
                    nc.sync.dma_start(
                        out=out_e[ot * 128:(ot + 1) * 128,
                                  tb * tblk:(tb + 1) * tblk],
                        in_=ob[:])

        # --- grouped pipeline: dequant(g0), then for each g: dequant(g+1)
        # interleaves (by engine independence) with gemm(g) ---------------
        groups = []
        o0 = 0
        for sz in CONFIG["groups"]:
            groups.append(list(range(o0, o0 + sz)))
            o0 += sz
        def body():
            if not CONFIG["weave"]:
                for g, ots in enumerate(groups):
                    for ot in ots:
                        dequant_otile(ot)
                    if g > 0:
                        gemm_group(groups[g - 1])
                gemm_group(groups[-1])
                return
            # weave: alternate one o-tile of dequant with a chunk of the
            # previous group's token blocks
            for g, ots in enumerate(groups):
                prev = groups[g - 1] if g > 0 else None
                nchunk = len(ots)
                for i, ot in enumerate(ots):
                    dequant_otile(ot)
                    if prev is not None:
                        t0 = (i * ntb) // nchunk
                        t1 = ((i + 1) * ntb) // nchunk
                        gemm_group(prev, range(t0, t1))
            gemm_group(groups[-1], range(ntb))

        if CONFIG.get("loop_n"):
            # timing variant: run the whole pipeline loop_n times inside the
            # NEFF (back-edge is a full barrier, so iterations are idempotent)
            with tc.For_i(0, CONFIG["loop_n"], 1):
                body()
        else:
            body()
        ctx.close()

    nc.compile()
    return nc


def _get_program():
    global _PROGRAM
    if _PROGRAM is None:
        _PROGRAM = _build_program()
    return _PROGRAM


def _shard_inputs(x, qweight, lut, bias):
    x = np.asarray(x, dtype=np.float16)
    qweight = np.asarray(qweight, dtype=np.int32)
    lut = np.asarray(lut, dtype=np.float16)
    bias = np.asarray(bias, dtype=np.float16)

    xt = x.reshape(T, IN)
    # i' = jj*128 + w  <->  i = 32w + jj ; xt_perm[i', t] = x[t, i]
    xt_perm = xt.reshape(T, 128, 32).transpose(2, 1, 0).reshape(IN, T)
    # re-tile to [tb, p, kt, u]: per-partition-contiguous slab DMAs
    tblk = CONFIG["tblk"]
    xt_perm = np.ascontiguousarray(
        xt_perm.reshape(KT, 128, T // tblk, tblk).transpose(2, 1, 0, 3))

    qw_pad = np.zeros((4, O_PAD, 128), np.int32)
    qw_pad[:, :O_FULL] = qweight[:4]
    lut_pad = np.zeros((O_PAD, 16), np.float16)
    lut_pad[:O_FULL] = lut
    bias_pad = np.zeros((O_PAD, 1), np.float16)
    bias_pad[:O_FULL, 0] = bias

    in_maps = []
    for c in range(NCORES):
        sl = slice(c * O_SH, (c + 1) * O_SH)
        in_maps.append({
            "qw": np.ascontiguousarray(qw_pad[:, sl]),
            "lut": np.ascontiguousarray(lut_pad[sl]),
            "bias": np.ascontiguousarray(bias_pad[sl]),
            "xt": xt_perm,
        })
    return in_maps


def _gather(results):
    full = np.concatenate([np.asarray(r["out"]) for r in results], axis=0)
    y = full[:O_FULL].T  # [T, O_FULL]
    return np.ascontiguousarray(y.reshape(2, 2048, O_FULL), dtype=np.float16)


def kernel(x, qweight, lut, bias, w_bits=4):
    from concourse.bass_utils import run_bass_kernel_spmd

    assert int(w_bits) == 4, f"kernel hardcodes w_bits=4, got {w_bits}"
    nc = _get_program()
    in_maps = _shard_inputs(x, qweight, lut, bias)
    res = run_bass_kernel_spmd(nc, in_maps, core_ids=list(range(NCORES)))
    return _gather(res.results)


def _time_nc(nc, in_maps, reps=5):
    """Min wall-clock (ns) of dispatching one NEFF exec of `nc` on 8 cores,
    inputs device-resident, donated zero output buffers made per rep."""
    import time
    import jax
    import jax.numpy as jnp
    from jax.sharding import Mesh, PartitionSpec, NamedSharding
    from jax.experimental.shard_map import shard_map
    import concourse.mybir as mybir
    from concourse.bass2jax import (_bass_exec_p, install_neuronx_cc_hook,
                                    partition_id_tensor)

    install_neuronx_cc_hook()
    n_cores = NCORES
    pid_name = nc.partition_id_tensor.name if nc.partition_id_tensor else None
    in_names, out_names, out_avals = [], [], []
    for alloc in nc.m.functions[0].allocations:
        if not isinstance(alloc, mybir.MemoryLocationSet):
            continue
        name = alloc.memorylocations[0].name
        if alloc.kind == "ExternalInput":
            if name != pid_name:
                in_names.append(name)
        elif alloc.kind == "ExternalOutput":
            out_names.append(name)
            out_avals.append(jax.core.ShapedArray(
                tuple(alloc.tensor_shape), mybir.dt.np(alloc.dtype)))
    n_params = len(in_names)
    n_outs = len(out_names)
    bind_in_names = list(in_names) + list(out_names)
    if pid_name is not None:
        bind_in_names.append(pid_name)

    def _body(*args):
        operands = list(args)
        if pid_name is not None:
            operands.append(partition_id_tensor())
        return tuple(_bass_exec_p.bind(
            *operands,
            out_avals=tuple(out_avals),
            in_names=tuple(bind_in_names),
            out_names=tuple(out_names),
            lowering_input_output_aliases=(),
            sim_require_finite=True,
            sim_require_nnan=True,
            nc=nc,
        ))

    devices = jax.devices()[:n_cores]
    mesh = Mesh(np.asarray(devices), ("core",))
    spec = PartitionSpec("core")
    sh = NamedSharding(mesh, spec)
    sharded = jax.jit(shard_map(
        _body, mesh=mesh,
        in_specs=(spec,) * (n_params + n_outs),
        out_specs=(spec,) * n_outs,
        check_rep=False),
        donate_argnums=tuple(range(n_params, n_params + n_outs)),
        keep_unused=True)
    gz = [(n_cores * a.shape[0], *a.shape[1:]) for a in out_avals]
    make_zeros = jax.jit(
        lambda: tuple(jnp.zeros(s_, a.dtype) for s_, a in zip(gz, out_avals)),
        out_shardings=tuple([sh] * n_outs))
    concat_in = [jax.device_put(
        np.concatenate([np.asarray(in_maps[c][nm]) for c in range(n_cores)],
                       axis=0), sh) for nm in in_names]
    out_arrs = sharded(*concat_in, *make_zeros())
    jax.block_until_ready(out_arrs)
    walls = []
    for _ in range(reps):
        z = make_zeros()
        jax.block_until_ready(z)
        t0 = time.perf_counter_ns()
        out_arrs = sharded(*concat_in, *z)
        jax.block_until_ready(out_arrs)
        walls.append(time.perf_counter_ns() - t0)
    results = [
        {nm: np.asarray(out_arrs[i]).reshape(n_cores, *out_avals[i].shape)[c]
         for i, nm in enumerate(out_names)}
        for c in range(n_cores)
    ]
    return walls, results


def run_timed(x, qweight, lut, bias, reps=5, pair=(16, 48)):
    """Return (y, walls_16, walls_48, per_exec_ns).

    Axon dispatch overhead is ~78ms/call and drifts between sessions, so
    device time is measured by the slope between two in-NEFF loop counts:
    per-exec = (min wall[48] - min wall[16]) / 32.
    """
    global _PROGRAM
    in_maps = _shard_inputs(x, qweight, lut, bias)

    CONFIG["loop_n"] = None
    _PROGRAM = None
    ncA = _get_program()
    _, results = _time_nc(ncA, in_maps, reps=1)

    CONFIG["loop_n"] = pair[0]
    _PROGRAM = None
    walls16, _ = _time_nc(_get_program(), in_maps, reps=reps)
    CONFIG["loop_n"] = pair[1]
    _PROGRAM = None
    walls48, _ = _time_nc(_get_program(), in_maps, reps=reps)
    CONFIG["loop_n"] = None
    _PROGRAM = None

    per_exec = (min(walls48) - min(walls16)) / (pair[1] - pair[0])
    return _gather(results), walls16, walls48, per_exec


def np_arr(x):
    return np.asarray(x)
